# revision 24
# baseline (speedup 1.0000x reference)
"""Trainium2 Bass kernel for nn_Embedding_loss (masked per-instance embedding loss).

Math: for each instance k with class c_k, over the (H,W) plane:
    cnt_k = sum(mask_k), s1_k = sum(emb[c_k] * mask_k), s2_k = sum(emb[c_k]^2 * mask_k)
Per-instance means/variances plus the tiny O(K^2) pairwise hinge term are
assembled on the host from the (s1, s2, cnt) triples.

Sharding: the 512*512 pixel plane is split across the 8 cores (32768 pixels
each).  Each core computes, for ALL K instances and all used classes u,
    out[k, u]       = sum_p mask_k[p] * E_u[p]        (s1 partial vs class u)
    out[k, U + u]   = sum_p mask_k[p] * E_u[p]^2      (s2 partial vs class u)
as one big PE contraction: 256 accumulating matmuls with the 128-pixel tile
of masks as the stationary operand (fp8) and [E | E^2] as the moving operand.
The host then picks column slot(c_k) per instance and sums partials over
cores.  This moves the multiply-reduce from VectorE/ScalarE (the 48us
baseline bottleneck) onto the otherwise-idle PE, and cuts HBM traffic from
6.8MB/core to ~5.2MB/core (masks are sent once per pixel instead of per
gathered instance; embedding planes are shared across instances).

E^2 is computed on-device (DVE/ScalarE alternate per chunk, overlapped with
the DMA stream).  fp8 quantization of embeddings moves the final loss by
~2e-5 relative - far inside tolerance.
"""

import os

import numpy as np

import concourse.bass as bass
import concourse.tile as tile
from concourse import mybir
from concourse.bass_utils import run_bass_kernel_spmd

N_CORES = 8
C, H, W = 80, 512, 512
K = 100
HWTOT = H * W  # 262144
P = 128  # SBUF partitions / matmul contraction
PIX = HWTOT // N_CORES  # 32768 pixels per core
NT = PIX // P  # 256 contraction tiles per core
CHUNK = 16  # tiles per pipeline chunk
NCH = NT // CHUNK  # 16 chunks

_NC_CACHE = {}
LAST_RESULT = None  # BassKernelResults of the most recent run (for test harness)

# fp8 DoubleRow matmul: contract 256 pixels per PE instruction (2 k-tiles).
# Needs mask cols padded to a 16B multiple (112) and u_pad = 16B multiple.
DOUBLE_ROW = bool(int(os.environ.get("KERNEL_DOUBLE_ROW", "0")))


def _split_sync(nc, max_w=1, max_u=1):
    """Walrus in this env accepts at most one sync wait/update per instruction;
    Tile's kernel-tail drain aggregates several. Split extras onto NoOps on the
    same engine (sequential waits on one queue are an AND, so semantics hold)."""
    ctr = 0
    for f in nc.m.functions:
        for bb in f.blocks:
            new = []
            for inst in bb.instructions:
                si = getattr(inst, "sync_info", None)
                waits = list(si.on_wait) if si is not None and si.on_wait else []
                updates = (
                    list(si.on_update) if si is not None and si.on_update else []
                )
                pre, post = [], []
                if len(waits) > max_w:
                    extra, keep = waits[:-max_w], waits[-max_w:]
                    si.on_wait = keep
                    for w in extra:
                        ctr += 1
                        nop = mybir.InstNoOp(name=f"syncsplit-w-{ctr}", ins=[], outs=[])
                        nop.engine = inst.engine
                        nop.sync_info = mybir.SyncInfo(on_wait=[w], on_update=[])
                        pre.append(nop)
                if len(updates) > max_u:
                    keep_u, extra_u = updates[:max_u], updates[max_u:]
                    si.on_update = keep_u
                    for u in extra_u:
                        ctr += 1
                        nop = mybir.InstNoOp(name=f"syncsplit-u-{ctr}", ins=[], outs=[])
                        nop.engine = inst.engine
                        nop.sync_info = mybir.SyncInfo(on_wait=[], on_update=[u])
                        post.append(nop)
                new.extend(pre)
                new.append(inst)
                new.extend(post)
            bb.instructions = new


def _build_program(u_pad, k_pad, dr):
    """One SPMD Bass program: PE-contract masks against [E | E^2] pixel tiles."""
    key = (u_pad, k_pad, dr)
    if key in _NC_CACHE:
        return _NC_CACHE[key]

    f8 = mybir.dt.float8e4
    nc = bass.Bass()
    masks8 = nc.declare_dram_parameter("masks8", [P, NT, k_pad], f8, isOutput=False)
    emb8 = nc.declare_dram_parameter("emb8", [P, NT, u_pad], f8, isOutput=False)
    stats = nc.declare_dram_parameter(
        "stats", [2, k_pad, 2 * u_pad], mybir.dt.float32, isOutput=True
    )
    k = k_pad

    # DMA chunk boundaries (tiles): small first for fast pipeline fill.
    # Each chunk costs one ~0.7us DIRECT2D trigger on the SP queue, so few,
    # growing chunks; squares run at small subchunks on Act/DVE so the PE
    # is never gated on a multi-us square.
    m_edges = [0, 8, 32, 64, 104, 152, 200, 256]  # mask chunks (SP queue)
    e_edges = [0, 8, 32, 96, 160, 256]  # E chunks (Act HWDGE queue)
    sq_edges = [0, 8, 16] + list(range(32, NT + 1, 16))
    N_DVE_SQ = 5  # first subchunks on DVE while Act is still issuing E DMAs

    with tile.TileContext(nc) as tc:
        with (
            tc.tile_pool(name="big", bufs=1) as big,
            tc.psum_pool(name="pp", bufs=1) as pp,
        ):
            mt = big.tile([P, NT, k], f8)
            ee2 = big.tile([P, 2, NT, u_pad], f8)
            st = big.tile([k, 2, 2 * u_pad], mybir.dt.float32)
            # Two accumulation groups (tile halves) so the first half's
            # PSUM->SBUF copy + output DMA overlap the second half's matmuls.
            psA = pp.tile([k, 2 * u_pad], mybir.dt.float32, tag="psA")
            psB = pp.tile([k, 2 * u_pad], mybir.dt.float32, tag="psB")

            # E chunks on the Act HWDGE queue, mask chunks alone on the SP
            # queue: masks pace the PE, so their triggers issue 2x denser.
            for lo, hi in zip(e_edges[:-1], e_edges[1:]):
                sl = slice(lo, hi)
                nc.scalar.dma_start(out=ee2[:, 0, sl, :], in_=emb8[:, sl, :])
            for lo, hi in zip(m_edges[:-1], m_edges[1:]):
                sl = slice(lo, hi)
                nc.sync.dma_start(out=mt[:, sl, :], in_=masks8[:, sl, :])

            # E^2 in 16-tile subchunks.  DVE takes the first N_DVE_SQ (the
            # Act sequencer is still issuing E DMA triggers and then pays the
            # one-off ACT_TABLE_LOAD), Act the rest.
            for i, (slo, shi) in enumerate(zip(sq_edges[:-1], sq_edges[1:])):
                sl = slice(slo, shi)
                if i < N_DVE_SQ or i % 3 == 0:
                    nc.vector.tensor_tensor(
                        out=ee2[:, 1, sl, :],
                        in0=ee2[:, 0, sl, :],
                        in1=ee2[:, 0, sl, :],
                        op=mybir.AluOpType.mult,
                    )
                else:
                    nc.scalar.activation(
                        out=ee2[:, 1, sl, :],
                        in_=ee2[:, 0, sl, :],
                        func=mybir.ActivationFunctionType.Square,
                    )

            HALF = NT // 2

            def mm_range(ps, lo, hi):
                if dr:
                    for t in range(lo, hi, 2):
                        nc.tensor.matmul(
                            ps[:, :],
                            lhsT=mt[:, t : t + 2, :],
                            rhs=ee2[:, :, t : t + 2, :].rearrange(
                                "p e t c -> p t e c"
                            ),
                            start=(t == lo),
                            stop=(t == hi - 2),
                            perf_mode=mybir.MatmulPerfMode.DoubleRow,
                        )
                else:
                    for t in range(lo, hi):
                        nc.tensor.matmul(
                            ps[:, :],
                            lhsT=mt[:, t, :],
                            rhs=ee2[:, :, t, :],
                            start=(t == lo),
                            stop=(t == hi - 1),
                        )

            mm_range(psA, 0, HALF)
            nc.vector.tensor_copy(out=st[:, 0, :], in_=psA)
            nc.sync.dma_start(out=stats[0], in_=st[:, 0, :])
            mm_range(psB, HALF, NT)
            nc.vector.tensor_copy(out=st[:, 1, :], in_=psB)
            nc.sync.dma_start(out=stats[1], in_=st[:, 1, :])

    _NC_CACHE[key] = nc
    return nc


def _enable_jax_compile_cache():
    try:
        import jax

        jax.config.update("jax_compilation_cache_dir", "/tmp/jax_neff_cache")
        jax.config.update("jax_persistent_cache_min_entry_size_bytes", -1)
        jax.config.update("jax_persistent_cache_min_compile_time_secs", 0.0)
    except Exception:
        pass
    # NEFF disk cache keyed on BIR bytes (deterministic serialization):
    # skip walrus recompiles across processes.
    try:
        import hashlib
        import shutil

        from concourse import bass2jax

        orig = bass2jax.compile_bir_kernel
        if getattr(orig, "_neff_cache_wrapped", False):
            return

        def cached_compile(bir_json, tmpdir, neff_name="file.neff"):
            h = hashlib.sha256(
                bir_json if isinstance(bir_json, bytes) else bir_json.encode()
            ).hexdigest()
            cpath = f"/tmp/neff_cache/{h}.neff"
            if os.path.exists(cpath):
                dst = os.path.join(tmpdir, neff_name)
                shutil.copy(cpath, dst)
                return dst
            out = orig(bir_json, tmpdir, neff_name=neff_name)
            os.makedirs("/tmp/neff_cache", exist_ok=True)
            shutil.copy(out, cpath)
            return out

        cached_compile._neff_cache_wrapped = True
        bass2jax.compile_bir_kernel = cached_compile
    except Exception:
        pass


def kernel(pred_emb, gt_objmask, gt_classes):
    global LAST_RESULT
    pred_emb = np.asarray(pred_emb)
    gt_objmask = np.asarray(gt_objmask)
    cls = np.clip(np.asarray(gt_classes).astype(np.int64), 0, C - 1)
    k = gt_objmask.shape[0]

    used = np.unique(cls)  # sorted used class ids
    slot = np.searchsorted(used, cls)  # instance -> slot in `used`
    u = len(used)
    if DOUBLE_ROW:
        u_pad = ((u + 15) // 16) * 16
        k_pad = ((k + 15) // 16) * 16
    else:
        u_pad = ((u + 3) // 4) * 4
        k_pad = k

    _enable_jax_compile_cache()
    nc = _build_program(u_pad, k_pad, DOUBLE_ROW)
    if not getattr(nc, "_sync_split_done", False):
        _split_sync(nc)  # CoreSim can't execute the bare NoOps; HW path only
        nc._sync_split_done = True

    f8 = mybir.dt.np(mybir.dt.float8e4)
    # emb8_used[s] = fp8 plane of used class s, flattened over pixels
    emb8_used = pred_emb[used].astype(f8).reshape(u, HWTOT)
    one_f8 = np.ones((), dtype=f8).view(np.uint8)  # bit pattern of fp8 1.0
    maskbits = gt_objmask.reshape(k, HWTOT)
    mask8 = (maskbits.astype(np.uint8) * one_f8).view(f8)
    cnt = np.count_nonzero(maskbits, axis=1).astype(np.float64)

    in_maps = []
    for c in range(N_CORES):
        lo, hi = c * PIX, (c + 1) * PIX
        # [k, NT, P] -> [P, NT, k_pad]
        mk = np.zeros((P, NT, k_pad), dtype=f8)
        mk[:, :, :k] = mask8[:, lo:hi].reshape(k, NT, P).transpose(2, 1, 0)
        em = np.zeros((P, NT, u_pad), dtype=f8)
        em[:, :, :u] = emb8_used[:, lo:hi].reshape(u, NT, P).transpose(2, 1, 0)
        in_maps.append({"masks8": mk, "emb8": em})

    core_ids = list(range(N_CORES))
    trace = bool(os.environ.get("KERNEL_TRACE"))
    res = run_bass_kernel_spmd(
        nc,
        in_maps,
        core_ids,
        trace=trace,
        trace_cores=core_ids if trace else None,
    )
    LAST_RESULT = res

    s1 = np.zeros(k, dtype=np.float64)
    s2 = np.zeros(k, dtype=np.float64)
    ar = np.arange(k)
    for c in range(N_CORES):
        stats = res.results[c]["stats"].astype(np.float64).sum(axis=0)  # (k, 2*u_pad)
        s1 += stats[ar, slot]
        s2 += stats[ar, u_pad + slot]

    has = cnt > 0
    safe = np.where(has, cnt, 1.0)
    mean = np.where(has, s1 / safe, 0.0)
    var = np.where(has, s2 / safe - mean * mean, 0.0)

    same = cls[:, None] == cls[None, :]
    upper = np.triu(np.ones((k, k), dtype=bool), 1)
    diff2 = (mean[:, None] - mean[None, :]) ** 2
    hinge = np.maximum(1.0 - diff2, 0.0)
    loss_inter = np.sum(np.where(same & upper, hinge, 0.0))
    loss_reg = np.mean(mean * mean)
    loss_intra = np.mean(var)
    loss = 1.0 * loss_inter + 1.0 * loss_reg + 1.0 * loss_intra
    return np.array([loss], dtype=np.float32)
